# revision 61
# baseline (speedup 1.0000x reference)
"""Trainium2 Bass kernel for nn_AttentionHead (single-head attention with
pre-softmax tril zeroing). B=8, S=2048, E=1024, H=64.

Sharding: data-parallel over batch - one batch element per NeuronCore,
no collectives. Each core computes, for its batch b:

  q = y@Wq + bq ; k' = x@(Wk/8) + (bk/8) ; v = x@Wv + bv
  scores[r, j] = q[r].k'[j] for j<=r, 0 for j>r      (tril PRE-softmax)
  attn = softmax(scores, -1)  -> masked entries contribute exp(0)=1
  out = attn @ v

v10 design - slab pipeline:
  - x,y host-cast to bf16, host-pre-transposed to [E, S], and laid out as
    4 s-range SLABS of shape [128, ECH*512] (partition-major) so qkv
    chunk i and attention column i unblock as soon as slab i lands
  - per slab: 16 projection matmuls (kv+q interleaved across 2 PSUM
    accumulators), evac kT(ACT)/qT(DVE)/vT(ACT or DVE), SBUF-SBUF DMA
    duplicates of kT/qT onto partitions 64:128, xbar DMA-transpose of
    the vT slab into natural layout with a fused ones-column
  - attention column c follows slab c: scores transposed (st[k, q]) in
    row-packed PAIRS via tile_position (0,0)/(64,0); exp on ACT into
    bf16; diagonal blocks exp only the live range and GpSimd fills
    masked cells with exp(0)=1; pv accumulates [v|1]^T @ ex in PSUM
  - each column's pv is copied to SBUF right after its last block so the
    bank recycles; the closed-form upper-triangle add (suffix sums of v,
    which need the last slab) + normalize + store run as a tail
  - two fp32 dummy matmuls at t=0 warm the PE HAM clock gate
"""

import numpy as np

import concourse.bass as bass
import concourse.mybir as mybir
from concourse.tile import TileContext

S, E, H = 2048, 1024, 64
SC = S // 128   # 16 s-chunks (key blocks)
ECH = E // 128  # 8 e-chunks
NQ = 4          # q-chunks (slabs) of 512
F32 = mybir.dt.float32
BF16 = mybir.dt.bfloat16
AF = mybir.ActivationFunctionType

_SPLIT_COUNTER = [0]


def _split_multi_waits(nc, ev_cap=1):
    """This container's walrus build accepts at most 1 sem-wait per
    instruction (2 on EventSemaphore); move excess waits onto EvSem
    instructions inserted just before, on the same engine."""
    for f in nc.m.functions:
        for bb in f.blocks:
            ins_list = bb.instructions
            need = False
            for ins in ins_list:
                si = ins.sync_info
                if si is None:
                    continue
                cap = 2 if isinstance(ins, mybir.InstEventSemaphore) else 1
                if len(si.on_wait) > cap:
                    need = True
                    break
            if not need:
                continue
            new_list = []
            for ins in ins_list:
                si = ins.sync_info
                cap = 2 if isinstance(ins, mybir.InstEventSemaphore) else 1
                if si is not None and len(si.on_wait) > cap:
                    waits = list(si.on_wait)
                    keep = waits[-cap:]
                    head = waits[:-cap]
                    for i in range(0, len(head), ev_cap):
                        _SPLIT_COUNTER[0] += 1
                        ev = mybir.InstEventSemaphore(
                            name=f"EVSPLIT-{_SPLIT_COUNTER[0]}",
                            engine=ins.engine,
                            ins=[],
                            outs=[],
                            sync_info=mybir.SyncInfo(
                                on_wait=head[i:i + ev_cap], on_update=[]
                            ),
                        )
                        nc.register_instruction(ev)
                        new_list.append(ev)
                    ins.sync_info = mybir.SyncInfo(
                        on_wait=keep, on_update=list(si.on_update)
                    )
                new_list.append(ins)
            bb.instructions = new_list


def _build():
    nc = bass.Bass()
    # x, y: [NQ slabs, 128, ECH*512] bf16 (host-packed, see kernel())
    x_ext = nc.declare_dram_parameter("x", [NQ, 128, ECH * 512], BF16,
                                      isOutput=False)
    y_ext = nc.declare_dram_parameter("y", [NQ, 128, ECH * 512], BF16,
                                      isOutput=False)
    wkv_ext = nc.declare_dram_parameter("wkv", [128, ECH * 128], BF16,
                                        isOutput=False)
    wq_ext = nc.declare_dram_parameter("wq", [128, ECH * H], BF16,
                                       isOutput=False)
    bq_ext = nc.declare_dram_parameter("bq", [H, 1], F32, isOutput=False)
    bk_ext = nc.declare_dram_parameter("bk", [H, 1], F32, isOutput=False)
    bv_ext = nc.declare_dram_parameter("bv", [H, 1], F32, isOutput=False)
    out_ext = nc.declare_dram_parameter("out", [S, H], F32, isOutput=True)

    with TileContext(nc) as tc:
        with (
            tc.tile_pool(name="consts", bufs=1) as consts,
            tc.tile_pool(name="bigT", bufs=1) as bigT,
            tc.tile_pool(name="expp", bufs=3) as expp,
            tc.tile_pool(name="outp", bufs=2) as outp,
        ):
            # ---- constants ----
            ident_f = consts.tile([128, 128], F32)
            nc.vector.memset(ident_f, 1.0)
            nc.gpsimd.affine_select(
                out=ident_f, in_=ident_f,
                pattern=[[-1, 128]], channel_multiplier=1, base=0,
                compare_op=mybir.AluOpType.is_equal, fill=0.0,
            )
            ident_bf = consts.tile([128, 128], BF16)
            nc.vector.memset(ident_bf, 1.0)
            nc.gpsimd.affine_select(
                out=ident_bf, in_=ident_bf,
                pattern=[[-1, 128]], channel_multiplier=1, base=0,
                compare_op=mybir.AluOpType.is_equal, fill=0.0,
            )

            # ---- PE warm-up: two fp32 matmuls (~3.4us) flip the HAM ----
            scr = consts.tile([128, 512], F32, tag="scr")
            nc.vector.memset(scr, 0.0)
            with tc.tile_pool(name="psW", bufs=1, space="PSUM") as psW:
                wm = psW.tile([128, 512], F32, tag="warm")
                for _ in range(2):
                    nc.tensor.matmul(wm, lhsT=ident_f, rhs=scr,
                                     start=True, stop=True)

            # ---- input slabs + weights; biases on SWDGE ----
            w_kv = consts.tile([128, ECH * 128], BF16, tag="w_kv")
            w_q = consts.tile([128, ECH * H], BF16, tag="w_q")
            bias_sb = {}
            for name, bext in (("q", bq_ext), ("k", bk_ext), ("v", bv_ext)):
                bs = consts.tile([H, 1], F32, tag=f"b_{name}",
                                 name=f"bias_{name}")
                nc.gpsimd.dma_start(out=bs, in_=bext[:, :])
                bias_sb[name] = bs

            xTs = [bigT.tile([128, ECH * 512], BF16, tag=f"xT{i}",
                             name=f"xT_{i}") for i in range(NQ)]
            yTs = [bigT.tile([128, ECH * 512], BF16, tag=f"yT{i}",
                             name=f"yT_{i}") for i in range(NQ)]
            # slab 0 split across both queues so kv can start earliest
            half0 = ECH * 256
            nc.sync.dma_start(out=xTs[0][:, 0:half0],
                              in_=x_ext[0][:, 0:half0])
            nc.scalar.dma_start(out=xTs[0][:, half0:],
                                in_=x_ext[0][:, half0:])
            nc.sync.dma_start(out=w_kv, in_=wkv_ext[:, :])
            nc.scalar.dma_start(out=w_q, in_=wq_ext[:, :])
            nc.sync.dma_start(out=yTs[0][:, 0:half0],
                              in_=y_ext[0][:, 0:half0])
            nc.scalar.dma_start(out=yTs[0][:, half0:],
                                in_=y_ext[0][:, half0:])
            for i in range(1, NQ):
                nc.sync.dma_start(out=xTs[i], in_=x_ext[i])
                nc.scalar.dma_start(out=yTs[i], in_=y_ext[i])

            # kT/qT duplicated on partitions 64:128 for row-packed scores
            qTd = bigT.tile([128, S], BF16, tag="qTd")
            kTd = bigT.tile([128, S], BF16, tag="kTd")
            vT = bigT.tile([H, S], BF16, tag="vT")
            v_aug = bigT.tile([128, SC * (H + 1)], BF16, tag="vaug")
            nc.gpsimd.memset(v_aug, 1.0)
            v_nat = bigT.tile([128, SC * H], BF16, tag="vnat")
            vsuf = []
            wins = [None]
            for c in range(NQ):
                va = consts.tile([H + 1, 1], F32, tag=f"vsuf{c}",
                                 name=f"vsuf_{c}")
                nc.vector.memset(va, 0.0)
                if c < NQ - 1:
                    nc.vector.memset(va[H:H + 1, :],
                                     float((NQ - 1 - c) * 512))
                vsuf.append(va)
            for g in (1, 2):
                wins.append(consts.tile([H, 1], F32, tag=f"win{g}",
                                        name=f"win_{g}"))

            sbns = []
            with (
                tc.tile_pool(name="psQ", bufs=1, space="PSUM") as psQ,
                tc.tile_pool(name="psE", bufs=1, space="PSUM") as psE,
            ):
                def finish(c):
                    # closed-form upper add + normalize + store col c
                    sbn = sbns[c]
                    if c < NQ - 1:
                        nc.vector.tensor_scalar_add(out=sbn, in0=sbn,
                                                    scalar1=vsuf[c])
                    pt4 = psE.tile([128, 4 * (H + 4)], BF16, tag="pt",
                                   bufs=1, name=f"pt4_{c}")
                    pt4v = pt4.rearrange("p (j h) -> p j h", h=H + 4)
                    for j4 in range(4):
                        nc.tensor.transpose(
                            pt4[:, j4 * (H + 4):j4 * (H + 4) + H + 1],
                            sbn[:, j4 * 128:(j4 + 1) * 128],
                            ident_bf[0:H + 1, 0:H + 1],
                        )
                    rcp4 = outp.tile([128, 4], F32, tag="rcp",
                                     name=f"rcp4_{c}")
                    nc.vector.reciprocal(
                        rcp4.rearrange("p (j o) -> p j o", o=1),
                        pt4v[:, :, H:H + 1])
                    of4 = outp.tile([128, 4 * H], F32, tag="of",
                                    name=f"of4_{c}")
                    of4v = of4.rearrange("p (j h) -> p j h", h=H)
                    for j4 in range(4):
                        nc.vector.tensor_scalar_mul(
                            out=of4v[:, j4, :], in0=pt4v[:, j4, 0:H],
                            scalar1=rcp4[:, j4:j4 + 1])
                    nc.sync.dma_start(
                        out=out_ext[c * 512:(c + 1) * 512, :].rearrange(
                            "(j p) h -> p j h", p=128),
                        in_=of4v)

                for i in range(NQ):
                    # ---- projections for slab i ----
                    kv_acc = psQ.tile([128, 512], F32, tag="kvacc", bufs=1,
                                      name=f"kvacc_{i}")
                    q_acc = psQ.tile([H, 512], F32, tag="qacc", bufs=1,
                                     name=f"qacc_{i}")
                    # slab 0: kv first (x lands before y), else interleave
                    order = ([("kv", e) for e in range(ECH)]
                             + [("q", e) for e in range(ECH)]) if i == 0 else \
                        [t for e in range(ECH) for t in (("kv", e), ("q", e))]
                    for kind, e in order:
                        if kind == "kv":
                            nc.tensor.matmul(
                                kv_acc,
                                lhsT=w_kv[:, e * 128:(e + 1) * 128],
                                rhs=xTs[i][:, e * 512:(e + 1) * 512],
                                start=(e == 0),
                                stop=(e == ECH - 1),
                            )
                        else:
                            nc.tensor.matmul(
                                q_acc,
                                lhsT=w_q[:, e * H:(e + 1) * H],
                                rhs=yTs[i][:, e * 512:(e + 1) * 512],
                                start=(e == 0),
                                stop=(e == ECH - 1),
                            )
                    sl = slice(i * 512, (i + 1) * 512)
                    nc.vector.tensor_scalar_add(
                        out=kTd[0:H, sl], in0=kv_acc[0:H, :],
                        scalar1=bias_sb["k"])
                    nc.vector.tensor_scalar_add(
                        out=qTd[0:H, sl], in0=q_acc, scalar1=bias_sb["q"])
                    nc.sync.dma_start(out=kTd[H:128, sl], in_=kTd[0:H, sl])
                    nc.sync.dma_start(out=qTd[H:128, sl], in_=qTd[0:H, sl])
                    nc.vector.tensor_scalar_add(
                        out=vT[:, sl], in0=kv_acc[H:128, :],
                        scalar1=bias_sb["v"])
                    nc.sync.dma_start(
                        out=v_nat.rearrange(
                            "p (j h) -> p j h", h=H
                        )[:, 4 * i:4 * i + 4, :],
                        in_=vT[:, sl], transpose=True,
                    )
                    nc.gpsimd.tensor_copy(
                        v_aug.rearrange(
                            "p (j h) -> p j h", h=H + 1
                        )[:, 4 * i:4 * i + 4, 0:H],
                        v_nat.rearrange(
                            "p (j h) -> p j h", h=H
                        )[:, 4 * i:4 * i + 4, :],
                    )
                    if i in (1, 2):
                        nc.vector.reduce_sum(
                            out=wins[i], in_=vT[:, sl],
                            axis=mybir.AxisListType.X)
                    elif i == 3:
                        nc.vector.reduce_sum(
                            out=vsuf[2][0:H, :], in_=vT[:, sl],
                            axis=mybir.AxisListType.X)
                        nc.vector.tensor_add(
                            out=vsuf[1][0:H, :], in0=vsuf[2][0:H, :],
                            in1=wins[2])
                        nc.vector.tensor_add(
                            out=vsuf[0][0:H, :], in0=vsuf[1][0:H, :],
                            in1=wins[1])

                    # ---- attention column i ----
                    c = i
                    pv = psE.tile([H + 1, 512], F32, tag="pv", bufs=2,
                                  name=f"pv_{c}")
                    nb = 4 * c + 4
                    for b2 in range(nb // 2):
                        st = [None, None]
                        ex = [None, None]
                        diag = (2 * b2) // 4 == c
                        for half in range(2):
                            b = 2 * b2 + half
                            st[half] = psE.tile([128, 512], F32, tag="st",
                                                bufs=3,
                                                name=f"st_{c}_{b2}_{half}")
                            lo = half * H
                            d0 = 128 * (b - 4 * c) if diag else 0
                            nc.tensor.matmul(
                                st[half][:, d0:],
                                lhsT=kTd[lo:lo + H, b * 128:(b + 1) * 128],
                                rhs=qTd[lo:lo + H,
                                        c * 512 + d0:(c + 1) * 512],
                                start=True,
                                stop=True,
                                tile_position=(lo, 0),
                            )
                        for half in range(2):
                            b = 2 * b2 + half
                            ex[half] = expp.tile([128, 512], BF16,
                                                 tag="expst", bufs=40,
                                                 name=f"ex_{c}_{b2}_{half}")
                            if diag:
                                d = b - 4 * c
                                nc.scalar.activation(
                                    out=ex[half][:, 128 * d:],
                                    in_=st[half][:, 128 * d:],
                                    func=AF.Exp)
                                w = 128 * (d + 1)
                                nc.gpsimd.affine_select(
                                    out=ex[half][:, 0:w],
                                    in_=ex[half][:, 0:w],
                                    pattern=[[1, w]], channel_multiplier=-1,
                                    base=-128 * d,
                                    compare_op=mybir.AluOpType.is_ge,
                                    fill=1.0,
                                )
                            else:
                                nc.scalar.activation(out=ex[half],
                                                     in_=st[half],
                                                     func=AF.Exp)
                        for half in range(2):
                            b = 2 * b2 + half
                            nc.tensor.matmul(
                                pv,
                                lhsT=v_aug[:, b * (H + 1):(b + 1) * (H + 1)],
                                rhs=ex[half],
                                start=(b == 0),
                                stop=(b == nb - 1),
                            )
                    # evacuate pv -> SBUF immediately (bank recycles);
                    # closed-form add + normalize happen in the tail
                    sbn = outp.tile([H + 1, 512], BF16, tag="sbn", bufs=4,
                                    name=f"sbn_{c}")
                    nc.vector.tensor_copy(sbn, pv)
                    sbns.append(sbn)

                # ---- finishes for cols 2,3 (0,1 were emitted after
                # col 2 so their DVE/DMA chains overlap col 3) ----
                for c in range(NQ):
                    finish(c)

    _split_multi_waits(nc)
    return nc


LAST_EXEC_TIME_NS = None
_CACHE = {}


def kernel(x, y, Wq, bq, Wk, bk, Wv, bv):
    """Full-input entry point: shards batch over 8 NeuronCores (one batch
    element per core), runs the Bass kernel, gathers the full output."""
    global LAST_EXEC_TIME_NS
    import os

    import ml_dtypes
    from concourse.bass_utils import run_bass_kernel_spmd

    if "nc" not in _CACHE:
        _CACHE["nc"] = _build()
    nc = _CACHE["nc"]

    bf = ml_dtypes.bfloat16
    x = np.asarray(x, np.float32)
    y = np.asarray(y, np.float32)

    # host-side weight packing: [128, ECH, 128] -> [128, ECH*128]
    wk8 = (np.asarray(Wk, np.float32) * 0.125).astype(bf).reshape(ECH, 128, H)
    wv2 = np.asarray(Wv, np.float32).astype(bf).reshape(ECH, 128, H)
    wkv = np.ascontiguousarray(
        np.concatenate([wk8, wv2], axis=2).transpose(1, 0, 2)
    ).reshape(128, ECH * 128)
    wq2 = np.ascontiguousarray(
        np.asarray(Wq, np.float32).astype(bf).reshape(ECH, 128, H)
        .transpose(1, 0, 2)
    ).reshape(128, ECH * H)
    bqc = np.ascontiguousarray(np.asarray(bq, np.float32).reshape(H, 1))
    bkc = np.ascontiguousarray(
        np.asarray(bk, np.float32).reshape(H, 1) * 0.125)
    bvc = np.ascontiguousarray(np.asarray(bv, np.float32).reshape(H, 1))

    def slabs(a):
        # [S, E] f32 -> bf16 [E, S] -> [NQ, 128, ECH*512] slab-major:
        # slab i, partition p, (e, s) -> a.T[e*128+p, i*512+s]
        t = a.astype(bf).T.reshape(ECH, 128, NQ, 512)
        return np.ascontiguousarray(
            t.transpose(2, 1, 0, 3).reshape(NQ, 128, ECH * 512))

    in_maps = []
    for b in range(8):
        in_maps.append({
            "x": slabs(x[b]), "y": slabs(y[b]),
            "wkv": wkv, "wq": wq2,
            "bq": bqc, "bk": bkc, "bv": bvc,
        })

    trace = bool(os.environ.get("ATTN_TRACE"))
    res = run_bass_kernel_spmd(nc, in_maps, core_ids=list(range(8)),
                               trace=trace)
    if trace:
        LAST_EXEC_TIME_NS = res.exec_time_ns
    return np.stack([res.results[i]["out"] for i in range(8)]).astype(
        np.float32)
